# revision 5
# baseline (speedup 1.0000x reference)
"""CosFace (AngularPenaltySMLoss) forward on 8 trn2 NeuronCores.

Reference computation (B=1024, D=512, C=100000):
    xn = l2norm_rows(input); wn = l2norm_rows(weight)
    cosine = xn @ wn.T                       # [B, C]  (output #2)
    tgt = cosine[b, label[b]]
    num = 30*(tgt - 0.4)
    denom = exp(num) + sum_c exp(30*cosine) - exp(30*tgt)
    loss = -mean(num - log(denom))           # scalar  (output #1)

Sharding: classes (C) split across the 8 cores, 12500 each, zero-padded to
12800 = 25 blocks x 512.  Each core computes its [1024, 12800] cosine shard
(bf16 GEMM, fp32 accumulate) and the partial exp-sum over its classes; the
[128, 8]-shaped per-row exp-sums are AllReduced across cores, and every core
computes the identical scalar loss.  Zero-pad classes contribute exactly
exp(0)=1 each to the exp-sum and are subtracted as a constant (2400).

The target logits are computed on-device from the (host-gathered) rows
weight[label] replicated to every core: tgt = (x.tw)/(|x||tw|) per row.

Per-core device program:
  prologue:  normalize x rows -> xn (bf16), PE-transpose -> xnT [512, 1024];
             per-row dot/norms of (x, tw) -> tgt [128, 8]
  25 blocks: DMA wT block [128, 4, 512] bf16 (host pre-transposed/blocked),
             class norms via Square + ones-matmul (broadcast across
             partitions), invw = 1/sqrt(ss + 1e-24),
             8 B-tiles: 4 accumulating matmuls -> psum [128, 512],
               cos = psum * invw -> DMA out; Exp(30*cos) row-accum
  epilogue:  AllReduce exp-sums, loss formula, partition-sum via matmul,
             loss scalar out.
"""

import numpy as np
import ml_dtypes

import concourse.bass as bass
import concourse.mybir as mybir
import concourse.tile as tile
from concourse import bacc
from concourse.masks import make_identity
from concourse.bass_utils import run_bass_kernel_spmd

F32 = mybir.dt.float32
BF16 = mybir.dt.bfloat16
AF = mybir.ActivationFunctionType
ALU = mybir.AluOpType

B, D, C = 1024, 512, 100000
N_CORES = 8
C_PER = C // N_CORES            # 12500 real classes per core
NBLK = 25                       # class blocks per core
BLK = 512                       # classes per block
C_PAD = NBLK * BLK              # 12800 (300 zero-pad classes per core)
PAD_TOTAL = (C_PAD - C_PER) * N_CORES   # 2400
MT = B // 128                   # 8 B-tiles of 128 rows
KT = D // 128                   # 4 contraction chunks
SCALE, MARGIN = 30.0, 0.4
EPS_SS = 1e-24                  # added to sum-of-squares before sqrt

_NC_CACHE = {}


def build_nc(n_iter=1):
    """Build + compile the 8-core SPMD bass module (cached).

    n_iter > 1 repeats the whole kernel body back-to-back inside one NEFF —
    used by test.py to measure steady-state HW exec time via the slope
    (T(n) - T(1)) / (n - 1), cancelling host/tunnel dispatch overhead.
    """
    if n_iter in _NC_CACHE:
        return _NC_CACHE[n_iter]

    nc = bacc.Bacc("TRN2", target_bir_lowering=False, debug=False,
                   num_devices=N_CORES)
    wt_d = nc.dram_tensor("wt", [NBLK, 128, KT, BLK], BF16,
                          kind="ExternalInput").ap()
    x_d = nc.dram_tensor("x", [B, D], F32, kind="ExternalInput").ap()
    tw_d = nc.dram_tensor("tw", [B, D], F32, kind="ExternalInput").ap()
    cos_d = nc.dram_tensor("cos", [B, C_PAD], F32, kind="ExternalOutput").ap()
    loss_d = nc.dram_tensor("loss", [1, 1], F32, kind="ExternalOutput").ap()

    with tile.TileContext(nc) as tc:
        for _ in range(n_iter):
            _body(tc, wt_d, x_d, tw_d, cos_d, loss_d)
    nc.compile()
    _NC_CACHE[n_iter] = nc
    return nc


def _body(tc, wt_d, x_d, tw_d, cos_d, loss_d):
    nc = tc.nc
    with (
        tc.tile_pool(name="const", bufs=1) as const,
        tc.tile_pool(name="xin", bufs=2) as xin,
        tc.tile_pool(name="pro", bufs=2) as pro,
        tc.tile_pool(name="wt", bufs=3) as wtp,
        tc.tile_pool(name="sq", bufs=3) as sqp,
        tc.tile_pool(name="nrm", bufs=2) as nrm,
        tc.tile_pool(name="cos", bufs=3) as cosp,
        tc.tile_pool(name="ej", bufs=2) as ejp,
        tc.tile_pool(name="fin", bufs=1) as fin,
        tc.tile_pool(name="ps_tp", bufs=2, space="PSUM") as ps_tp,
        tc.tile_pool(name="ps_ss", bufs=2, space="PSUM") as ps_ss,
        tc.tile_pool(name="ps_c", bufs=3, space="PSUM") as ps_c,
        tc.tile_pool(name="ps_l", bufs=1, space="PSUM") as ps_l,
        tc.tile_pool(name="dram", bufs=2, space="DRAM") as dram,
    ):
        ident = const.tile([128, 128], BF16)
        make_identity(nc, ident[:])
        ones128 = const.tile([128, 128], BF16)
        nc.vector.memset(ones128[:], 1.0)
        ones_f32 = const.tile([128, 1], F32)
        nc.vector.memset(ones_f32[:], 1.0)
        eps_t = const.tile([128, 1], F32)
        nc.vector.memset(eps_t[:], EPS_SS)

        # xnT[k]: [128 (D chunk), 1024 (B)] bf16, stationary operands
        xnT = [const.tile([128, B], BF16, name=f"xnT{k}") for k in range(KT)]
        # tgt[128, MT]: target cosine per row
        tgt = fin.tile([128, MT], F32)
        # es_parts[m]: per-block exp row-sums
        es_parts = [fin.tile([128, NBLK], F32, name=f"es{m}") for m in range(MT)]

        # ---------------- prologue: xn, xnT, tgt ----------------
        for m in range(MT):
            rows = slice(m * 128, (m + 1) * 128)
            x_t = xin.tile([128, D], F32)
            nc.sync.dma_start(out=x_t[:], in_=x_d[rows, :])
            tw_t = xin.tile([128, D], F32)
            nc.sync.dma_start(out=tw_t[:], in_=tw_d[rows, :])

            sjunk = pro.tile([128, D], F32)
            ssx = pro.tile([128, 1], F32)
            nc.scalar.activation(out=sjunk[:], in_=x_t[:], func=AF.Square,
                                 accum_out=ssx[:])
            invx = pro.tile([128, 1], F32)
            nc.scalar.activation(out=invx[:], in_=ssx[:], func=AF.Sqrt,
                                 bias=eps_t[:])
            nc.vector.reciprocal(out=invx[:], in_=invx[:])

            sst = pro.tile([128, 1], F32)
            nc.scalar.activation(out=sjunk[:], in_=tw_t[:], func=AF.Square,
                                 accum_out=sst[:])
            invt = pro.tile([128, 1], F32)
            nc.scalar.activation(out=invt[:], in_=sst[:], func=AF.Sqrt,
                                 bias=eps_t[:])
            nc.vector.reciprocal(out=invt[:], in_=invt[:])

            # per-row dot(x, tw)
            prod = pro.tile([128, D], F32)
            nc.vector.tensor_mul(prod[:], x_t[:], tw_t[:])
            dot = pro.tile([128, 1], F32)
            nc.scalar.activation(out=prod[:], in_=prod[:], func=AF.Copy,
                                 accum_out=dot[:])
            # tgt[:, m] = dot * invx * invt
            nc.vector.scalar_tensor_tensor(
                out=tgt[:, m:m + 1], in0=dot[:], scalar=invx[:], in1=invt[:],
                op0=ALU.mult, op1=ALU.mult)

            # xn (bf16) and its transpose into xnT
            xn_t = pro.tile([128, D], BF16)
            nc.vector.tensor_scalar_mul(xn_t[:], x_t[:], invx[:])
            for k in range(KT):
                pt = ps_tp.tile([128, 128], BF16)
                nc.tensor.transpose(pt[:], xn_t[:, k * 128:(k + 1) * 128],
                                    ident[:])
                nc.scalar.copy(out=xnT[k][:, m * 128:(m + 1) * 128],
                               in_=pt[:])

        # ---------------- main loop over class blocks ----------------
        for j in range(NBLK):
            wt_t = wtp.tile([128, KT, BLK], BF16)
            nc.sync.dma_start(out=wt_t[:], in_=wt_d[j])

            # class sum-of-squares, broadcast to all 128 partitions
            ss = ps_ss.tile([128, BLK], F32)
            for k in range(KT):
                sq = sqp.tile([128, BLK], BF16)
                nc.scalar.activation(out=sq[:], in_=wt_t[:, k, :],
                                     func=AF.Square)
                nc.tensor.matmul(ss[:], ones128[:], sq[:],
                                 start=(k == 0), stop=(k == KT - 1))
            invw = nrm.tile([128, BLK], F32)
            nc.scalar.activation(out=invw[:], in_=ss[:], func=AF.Sqrt,
                                 bias=eps_t[:])
            nc.vector.reciprocal(out=invw[:], in_=invw[:])

            for m in range(MT):
                pc = ps_c.tile([128, BLK], F32)
                for k in range(KT):
                    nc.tensor.matmul(pc[:],
                                     xnT[k][:, m * 128:(m + 1) * 128],
                                     wt_t[:, k, :],
                                     start=(k == 0), stop=(k == KT - 1))
                cos_t = cosp.tile([128, BLK], F32)
                nc.vector.tensor_mul(cos_t[:], pc[:], invw[:])
                nc.sync.dma_start(
                    out=cos_d[m * 128:(m + 1) * 128, j * BLK:(j + 1) * BLK],
                    in_=cos_t[:])
                ej = ejp.tile([128, BLK], BF16)
                nc.scalar.activation(out=ej[:], in_=cos_t[:], func=AF.Exp,
                                     scale=SCALE,
                                     accum_out=es_parts[m][:, j:j + 1])

        # ---------------- epilogue: allreduce + loss ----------------
        s_loc = fin.tile([128, MT], F32)
        for m in range(MT):
            nc.vector.reduce_sum(out=s_loc[:, m:m + 1], in_=es_parts[m][:],
                                 axis=mybir.AxisListType.X)
        b_in = dram.tile([128, MT], F32)
        b_out = dram.tile([128, MT], F32)
        nc.sync.dma_start(out=b_in[:], in_=s_loc[:])
        nc.gpsimd.collective_compute(
            "AllReduce", ALU.add,
            replica_groups=[list(range(N_CORES))],
            ins=[b_in[:].opt()], outs=[b_out[:].opt()])
        s_tot = fin.tile([128, MT], F32)
        nc.sync.dma_start(out=s_tot[:], in_=b_out[:])

        # num = SCALE * (tgt - MARGIN)
        num = fin.tile([128, MT], F32)
        nc.vector.tensor_scalar(out=num[:], in0=tgt[:], scalar1=MARGIN,
                                scalar2=SCALE, op0=ALU.subtract, op1=ALU.mult)
        e1 = fin.tile([128, MT], F32)
        nc.scalar.activation(out=e1[:], in_=num[:], func=AF.Exp)
        e2 = fin.tile([128, MT], F32)
        nc.scalar.activation(out=e2[:], in_=tgt[:], func=AF.Exp, scale=SCALE)
        # denom = e1 + (s_tot - PAD_TOTAL - e2)
        den = fin.tile([128, MT], F32)
        nc.vector.tensor_scalar(out=den[:], in0=s_tot[:],
                                scalar1=float(PAD_TOTAL), scalar2=None,
                                op0=ALU.subtract)
        nc.vector.tensor_sub(den[:], den[:], e2[:])
        nc.vector.tensor_add(den[:], den[:], e1[:])
        ln = fin.tile([128, MT], F32)
        nc.scalar.activation(out=ln[:], in_=den[:], func=AF.Ln)
        lossv = fin.tile([128, MT], F32)
        nc.vector.tensor_sub(lossv[:], num[:], ln[:])
        lrow = fin.tile([128, 1], F32)
        nc.vector.reduce_sum(out=lrow[:], in_=lossv[:],
                             axis=mybir.AxisListType.X)
        pl = ps_l.tile([1, 1], F32)
        nc.tensor.matmul(pl[:], lrow[:], ones_f32[:], start=True, stop=True)
        lsb = fin.tile([1, 1], F32)
        nc.scalar.activation(out=lsb[:], in_=pl[:], func=AF.Copy,
                             scale=-1.0 / B)
        nc.sync.dma_start(out=loss_d[:], in_=lsb[:])


def make_in_maps(input, label, weight):
    """Host-side sharding/layout prep (pure data movement + dtype casts)."""
    x = np.ascontiguousarray(input, dtype=np.float32)
    tw = np.ascontiguousarray(weight[label], dtype=np.float32)  # [B, D] gather
    in_maps = []
    for i in range(N_CORES):
        ws = weight[i * C_PER:(i + 1) * C_PER].astype(np.float32)
        wsp = np.zeros((C_PAD, D), np.float32)
        wsp[:C_PER] = ws
        # blocked transpose: wt[j, p, k, c] = wsp[j*BLK + c, k*128 + p]
        wtb = np.ascontiguousarray(
            wsp.T.reshape(KT, 128, NBLK, BLK).transpose(2, 1, 0, 3)
        ).astype(ml_dtypes.bfloat16)
        in_maps.append({"wt": wtb, "x": x, "tw": tw})
    return in_maps


def assemble(results):
    """Gather per-core outputs into the reference's (loss, cosine) tuple."""
    cos = np.concatenate(
        [results[i]["cos"][:, :C_PER] for i in range(N_CORES)], axis=1)
    loss = np.float32(results[0]["loss"][0, 0])
    return (loss, cos)


def kernel(input, label, weight):
    nc = build_nc()
    in_maps = make_in_maps(input, label, weight)
    res = run_bass_kernel_spmd(nc, in_maps, core_ids=list(range(N_CORES)))
    return assemble(res.results)


# revision 22
# speedup vs baseline: 2.4561x; 2.4561x over previous
"""CosFace (AngularPenaltySMLoss) forward on 8 trn2 NeuronCores.

Reference computation (B=1024, D=512, C=100000):
    xn = l2norm_rows(input); wn = l2norm_rows(weight)
    cosine = xn @ wn.T                       # [B, C]  (output #2)
    tgt = cosine[b, label[b]]
    num = 30*(tgt - 0.4)
    denom = exp(num) + sum_c exp(30*cosine) - exp(30*tgt)
    loss = -mean(num - log(denom))           # scalar  (output #1)

Sharding (vocab/tensor parallel, per the hint): classes split across the 8
cores, 12500 each, zero-padded to 12800 = 25 blocks x 512.  Each core
computes its [1024, 12800] cosine shard (bf16 GEMM, fp32 PSUM accumulate)
and its partial per-row exp-sums; the [128, 8] exp-sums are AllReduced
across cores and every core computes the identical scalar loss.  Zero-pad
classes contribute exactly exp(0)=1 each and are subtracted as a constant.

Key device-side structure (per core):
  - Host ships the weight shard pre-transposed and block-packed
    ([25, 128, 4, 512] bf16) plus x pre-transposed ([4, 128, 1024] bf16):
    pure layout prep.  The whole weight shard stays resident in SBUF
    (100KB/partition), loaded once.
  - One activation-table set (natural_log_exp_and_others: Copy/Square/
    Ln/Exp) is pinned up front; rsqrt is computed as Exp(-0.5*Ln(x)), so
    the ScalarE never pays the ~2.7us table-switch cost.
  - Class norms: squares on DVE, partition-sum via an all-ones matmul
    (which also broadcasts the result to all 128 partitions), then
    invw = Exp(-0.5*Ln(ss + eps)) directly in bf16.
  - Main loop (m outer, 25 class blocks inner): 4 accumulating matmuls
    per (m, block) with the xT chunk stationary; one DVE
    scalar_tensor_tensor fuses 1/|x| (per-partition) and 1/|w| (per-class)
    into the PSUM->SBUF copy, writing bf16 cosine rows.
  - Per 5-block group: one 2560-wide DMA out and one 2560-wide Exp with
    accumulate (amortizing the ScalarE's 352-cycle fixed cost); m=0 is
    interleaved with the norm prepass so the PE stays busy while the
    weight DMAs stream in.
  - Epilogue: AllReduce the [128, 8] exp-sums, CosFace loss formula, and
    a ones-matmul partition reduction for the final mean.

The target logits are computed on-device from the host-gathered rows
weight[label] replicated to every core: tgt = (x.tw)/(|x||tw|) per row.
"""

import numpy as np
import ml_dtypes

import concourse.bass as bass
import concourse.mybir as mybir
import concourse.tile as tile
from concourse import bacc
from concourse.masks import make_identity
from concourse.bass_utils import run_bass_kernel_spmd

F32 = mybir.dt.float32
BF16 = mybir.dt.bfloat16
AF = mybir.ActivationFunctionType
ALU = mybir.AluOpType

B, D, C = 1024, 512, 100000
N_CORES = 8
C_PER = C // N_CORES            # 12500 real classes per core
NBLK = 25                       # class blocks per core
BLK = 512                       # classes per block
C_PAD = NBLK * BLK              # 12800 (300 zero-pad classes per core)
PAD_TOTAL = (C_PAD - C_PER) * N_CORES   # 2400
MT = B // 128                   # 8 B-tiles of 128 rows
KT = D // 128                   # 4 contraction chunks
SCALE, MARGIN = 30.0, 0.4
EPS_SS = 1e-24                  # added to sum-of-squares before sqrt
COS_DT = BF16                   # on-device cosine output dtype (host upcasts)

_NC_CACHE = {}

# Bumped on every kernel change: feeds the "salt" input's shape, which makes
# the lowered HLO (and therefore the neuronx compile-cache key) unique per
# kernel version.  Without it, stale NEFFs are served for same-shaped builds
# (the BIR itself is not part of the cache key).
VERSION = 9


def _salt_dim(n_iter):
    return VERSION * 64 + n_iter


def build_nc(n_iter=1):
    """Build + compile the 8-core SPMD bass module (cached).

    n_iter > 1 repeats the whole kernel body back-to-back inside one NEFF —
    used by test.py to measure steady-state HW exec time via the slope
    (T(n) - T(1)) / (n - 1), cancelling host/tunnel dispatch overhead.
    """
    if n_iter in _NC_CACHE:
        return _NC_CACHE[n_iter]

    nc = bacc.Bacc("TRN2", target_bir_lowering=False, debug=False,
                   num_devices=N_CORES)
    wt_d = nc.dram_tensor("wt", [NBLK, 128, KT, BLK], BF16,
                          kind="ExternalInput").ap()
    xt_d = nc.dram_tensor("xt", [KT, 128, B], BF16,
                          kind="ExternalInput").ap()
    x_d = nc.dram_tensor("x", [B, D], BF16, kind="ExternalInput").ap()
    tw_d = nc.dram_tensor("tw", [B, D], BF16, kind="ExternalInput").ap()
    cos_d = nc.dram_tensor("cos", [B, C_PAD], COS_DT,
                           kind="ExternalOutput").ap()
    loss_d = nc.dram_tensor("loss", [1, 1], F32, kind="ExternalOutput").ap()
    salt_d = nc.dram_tensor("salt", [1, _salt_dim(n_iter)], F32,
                            kind="ExternalInput").ap()

    with tile.TileContext(nc) as tc:
        with tc.tile_pool(name="salt", bufs=1) as saltp:
            salt_t = saltp.tile([1, _salt_dim(n_iter)], F32)
            nc.gpsimd.dma_start(out=salt_t[:], in_=salt_d[:])
        for _ in range(n_iter):
            _body(tc, wt_d, xt_d, x_d, tw_d, cos_d, loss_d)
    nc.compile()
    _NC_CACHE[n_iter] = nc
    return nc


def _body(tc, wt_d, xt_d, x_d, tw_d, cos_d, loss_d):
    nc = tc.nc
    from concourse.hw_specs import get_activation_tables
    act_sets = list(get_activation_tables(nc.m.arch))
    nlx_id = act_sets.index("natural_log_exp_and_others")

    with (
        tc.tile_pool(name="const", bufs=1) as const,
        tc.tile_pool(name="wt", bufs=NBLK) as wtp,
        tc.tile_pool(name="invw", bufs=NBLK) as invwp,
        tc.tile_pool(name="fin", bufs=1) as fin,
        tc.tile_pool(name="ps_ss", bufs=2, space="PSUM") as ps_ss,
        tc.tile_pool(name="ps_l", bufs=1, space="PSUM") as ps_l,
        tc.tile_pool(name="dram", bufs=2, space="DRAM") as dram,
    ):
        # Pin the one activation table set (Copy/Square/Ln/Exp) used by the
        # ENTIRE kernel: rsqrt is computed as Exp(-0.5*Ln(x)), so no sqrt
        # set is ever needed and the ~2.7us/set-switch cost is paid once.
        nc.scalar.add_instruction(mybir.InstLoadActFuncSet(
            name=nc.get_next_instruction_name(), act_func_set_id=nlx_id,
            ins=[], outs=[]))

        ones128 = const.tile([128, 128], BF16)
        nc.vector.memset(ones128[:], 1.0)
        ones_f32 = const.tile([128, 1], F32)
        nc.vector.memset(ones_f32[:], 1.0)
        eps_t = const.tile([128, 1], F32)
        nc.vector.memset(eps_t[:], EPS_SS)

        # xT[k]: [128 (D chunk), 1024 (B)] bf16 — RAW x columns, transposed
        # on the host; 1/|x_row| is folded into the cosine epilogue.
        xT = [const.tile([128, B], BF16, name=f"xT{k}") for k in range(KT)]
        for k in range(KT):
            nc.sync.dma_start(out=xT[k][:], in_=xt_d[k])
        tgt = fin.tile([128, MT], F32)          # target cosine per row
        es8 = fin.tile([128, MT, NGRP], F32)    # exp-sum partials per (m, g)
        invx = fin.tile([128, MT], F32)         # 1/|x_row|, [part, m]

        wts = [wtp.tile([128, KT, BLK], BF16, name=f"wt{j}", tag="wt")
               for j in range(NBLK)]
        invw = [invwp.tile([128, BLK], BF16, name=f"invw{j}", tag="invw")
                for j in range(NBLK)]

        # ---------------- DMAs + prologue + main loop ----------------
        with (
            tc.tile_pool(name="pxin", bufs=1) as pxin,
            tc.tile_pool(name="pro", bufs=2) as pro,
            tc.tile_pool(name="sq", bufs=3) as sqp,
            tc.tile_pool(name="nrm", bufs=2) as nrm,
            tc.tile_pool(name="cosg", bufs=3) as cosgp,
            tc.tile_pool(name="ej", bufs=1) as ejp,
            tc.tile_pool(name="ps_c", bufs=3, space="PSUM") as ps_c,
        ):
            # DMA order: wt_0 first (gates the first norm matmul), then the
            # x transposes (gate the first main matmul), then the rest.
            nc.sync.dma_start(out=wts[0][:], in_=wt_d[0])
            for k in range(KT):
                nc.sync.dma_start(out=xT[k][:], in_=xt_d[k])
            xall = pxin.tile([128, MT, D], BF16)
            twall = pxin.tile([128, MT, D], BF16)
            nc.gpsimd.dma_start(out=xall[:],
                                in_=x_d.rearrange("(m p) d -> p m d", p=128))
            for j in range(1, 5):
                nc.sync.dma_start(out=wts[j][:], in_=wt_d[j])
            nc.gpsimd.dma_start(out=twall[:],
                                in_=tw_d.rearrange("(m p) d -> p m d", p=128))
            for j in range(5, NBLK):
                nc.sync.dma_start(out=wts[j][:], in_=wt_d[j])

            def prepass(j):
                # class sum-of-squares broadcast to all partitions via
                # ones-matmul; invw_j = Exp(-0.5*Ln(ss+eps)) in bf16
                ss = ps_ss.tile([128, BLK], F32)
                for k in range(KT):
                    sq = sqp.tile([128, BLK], BF16)
                    nc.vector.tensor_mul(sq[:], wts[j][:, k, :],
                                         wts[j][:, k, :])
                    nc.tensor.matmul(ss[:], ones128[:], sq[:],
                                     start=(k == 0), stop=(k == KT - 1))
                lss = nrm.tile([128, BLK], F32)
                nc.scalar.activation(out=lss[:], in_=ss[:], func=AF.Ln,
                                     bias=eps_t[:])
                nc.scalar.activation(out=invw[j][:], in_=lss[:], func=AF.Exp,
                                     scale=-0.5)

            def main_mg(m, g):
                cosg = cosgp.tile([128, GRP * BLK], COS_DT)
                for jj in range(GRP):
                    j = g * GRP + jj
                    pc = ps_c.tile([128, BLK], F32)
                    for k in range(KT):
                        nc.tensor.matmul(pc[:],
                                         xT[k][:, m * 128:(m + 1) * 128],
                                         wts[j][:, k, :],
                                         start=(k == 0), stop=(k == KT - 1))
                    # cosine = (raw_dot * invx_row) * invw_class
                    nc.vector.scalar_tensor_tensor(
                        out=cosg[:, jj * BLK:(jj + 1) * BLK], in0=pc[:],
                        scalar=invx[:, m:m + 1], in1=invw[j][:],
                        op0=ALU.mult, op1=ALU.mult)
                nc.sync.dma_start(
                    out=cos_d[m * 128:(m + 1) * 128,
                              g * GRP * BLK:(g + 1) * GRP * BLK],
                    in_=cosg[:])
                ej = ejp.tile([128, GRP * BLK], BF16)
                nc.scalar.activation(out=ej[:], in_=cosg[:], func=AF.Exp,
                                     scale=SCALE,
                                     accum_out=es8[:, m, g:g + 1])

            # group 0 prepass first so invw_0 is ready ASAP
            for jj in range(GRP):
                prepass(jj)

            # x norms: ACT Square+accum (Square is in the pinned table
            # set), then invx = Exp(-0.5 * Ln(ssx + eps)).
            ssx = pro.tile([128, MT], F32)
            for m in range(MT):
                sjunk = pro.tile([128, D], BF16)
                nc.scalar.activation(out=sjunk[:], in_=xall[:, m, :],
                                     func=AF.Square,
                                     accum_out=ssx[:, m:m + 1])
            lssx = pro.tile([128, MT], F32)
            nc.scalar.activation(out=lssx[:], in_=ssx[:], func=AF.Ln,
                                 bias=eps_t[:])
            nc.scalar.activation(out=invx[:], in_=lssx[:], func=AF.Exp,
                                 scale=-0.5)

            # head: m=0 interleaved with the remaining prepass groups
            main_mg(0, 0)
            for g in range(1, NGRP):
                for jj in range(GRP):
                    prepass(g * GRP + jj)
                main_mg(0, g)

            # target-logit path (needs tw; off the critical path)
            sst = pro.tile([128, MT], F32)
            dots = pro.tile([128, MT], F32)
            for m in range(MT):
                sjunk = pro.tile([128, D], BF16)
                nc.scalar.activation(out=sjunk[:], in_=twall[:, m, :],
                                     func=AF.Square,
                                     accum_out=sst[:, m:m + 1])
            nc.vector.tensor_mul(twall[:], xall[:], twall[:])  # in-place prod
            for m in range(MT):
                sjunk = pro.tile([128, D], BF16)
                nc.scalar.activation(out=sjunk[:], in_=twall[:, m, :],
                                     func=AF.Copy,
                                     accum_out=dots[:, m:m + 1])
            lsst = pro.tile([128, MT], F32)
            nc.scalar.activation(out=lsst[:], in_=sst[:], func=AF.Ln,
                                 bias=eps_t[:])
            invt = pro.tile([128, MT], F32)
            nc.scalar.activation(out=invt[:], in_=lsst[:], func=AF.Exp,
                                 scale=-0.5)
            # tgt = dots * invx * invt
            nc.vector.tensor_mul(tgt[:], dots[:], invx[:])
            nc.vector.tensor_mul(tgt[:], tgt[:], invt[:])

            for m in range(1, MT):
                for g in range(NGRP):
                    main_mg(m, g)

        # ---------------- epilogue: allreduce + loss ----------------
        s_loc = fin.tile([128, MT], F32)
        for m in range(MT):
            nc.vector.reduce_sum(out=s_loc[:, m:m + 1], in_=es8[:, m, :],
                                 axis=mybir.AxisListType.X)
        b_in = dram.tile([128, MT], F32)
        b_out = dram.tile([128, MT], F32)
        nc.sync.dma_start(out=b_in[:], in_=s_loc[:])
        nc.gpsimd.collective_compute(
            "AllReduce", ALU.add,
            replica_groups=[list(range(N_CORES))],
            ins=[b_in[:].opt()], outs=[b_out[:].opt()])
        s_tot = fin.tile([128, MT], F32)
        nc.sync.dma_start(out=s_tot[:], in_=b_out[:])

        # num = SCALE * (tgt - MARGIN)
        num = fin.tile([128, MT], F32)
        nc.vector.tensor_scalar(out=num[:], in0=tgt[:], scalar1=MARGIN,
                                scalar2=SCALE, op0=ALU.subtract, op1=ALU.mult)
        e1 = fin.tile([128, MT], F32)
        nc.scalar.activation(out=e1[:], in_=num[:], func=AF.Exp)
        e2 = fin.tile([128, MT], F32)
        nc.scalar.activation(out=e2[:], in_=tgt[:], func=AF.Exp, scale=SCALE)
        # denom = e1 + (s_tot - PAD_TOTAL - e2)
        den = fin.tile([128, MT], F32)
        nc.vector.tensor_scalar(out=den[:], in0=s_tot[:],
                                scalar1=float(PAD_TOTAL), scalar2=None,
                                op0=ALU.subtract)
        nc.vector.tensor_sub(den[:], den[:], e2[:])
        nc.vector.tensor_add(den[:], den[:], e1[:])
        ln = fin.tile([128, MT], F32)
        nc.scalar.activation(out=ln[:], in_=den[:], func=AF.Ln)
        lossv = fin.tile([128, MT], F32)
        nc.vector.tensor_sub(lossv[:], num[:], ln[:])
        lrow = fin.tile([128, 1], F32)
        nc.vector.reduce_sum(out=lrow[:], in_=lossv[:],
                             axis=mybir.AxisListType.X)
        pl = ps_l.tile([1, 1], F32)
        nc.tensor.matmul(pl[:], lrow[:], ones_f32[:], start=True, stop=True)
        lsb = fin.tile([1, 1], F32)
        nc.scalar.activation(out=lsb[:], in_=pl[:], func=AF.Copy,
                             scale=-1.0 / B)
        nc.sync.dma_start(out=loss_d[:], in_=lsb[:])


def make_in_maps(input, label, weight, n_iter=1):
    """Host-side sharding/layout prep (pure data movement + dtype casts)."""
    x = np.ascontiguousarray(input).astype(ml_dtypes.bfloat16)
    xt = np.ascontiguousarray(
        np.asarray(input, np.float32).T.reshape(KT, 128, B)
    ).astype(ml_dtypes.bfloat16)
    tw = np.ascontiguousarray(weight[label]).astype(ml_dtypes.bfloat16)
    salt = np.zeros((1, _salt_dim(n_iter)), np.float32)
    in_maps = []
    for i in range(N_CORES):
        ws = weight[i * C_PER:(i + 1) * C_PER].astype(np.float32)
        wsp = np.zeros((C_PAD, D), np.float32)
        wsp[:C_PER] = ws
        # blocked transpose: wt[j, p, k, c] = wsp[j*BLK + c, k*128 + p]
        wtb = np.ascontiguousarray(
            wsp.T.reshape(KT, 128, NBLK, BLK).transpose(2, 1, 0, 3)
        ).astype(ml_dtypes.bfloat16)
        in_maps.append({"wt": wtb, "xt": xt, "x": x, "tw": tw,
                        "salt": salt})
    return in_maps


def assemble(results):
    """Gather per-core outputs into the reference's (loss, cosine) tuple."""
    cos = np.concatenate(
        [results[i]["cos"][:, :C_PER].astype(np.float32)
         for i in range(N_CORES)], axis=1)
    loss = np.float32(results[0]["loss"][0, 0])
    return (loss, cos)


def kernel(input, label, weight):
    nc = build_nc()
    in_maps = make_in_maps(input, label, weight)
    last_err = None
    for attempt in range(3):
        try:
            res = run_bass_kernel_spmd(nc, in_maps,
                                       core_ids=list(range(N_CORES)))
            return assemble(res.results)
        except Exception as e:  # transient tunnel/device hiccups: retry
            last_err = e
            import time
            time.sleep(20.0 * (attempt + 1))
    raise last_err
